# revision 7
# baseline (speedup 1.0000x reference)
"""Trainium2 Bass kernel for nn_MoELayer_15934328668398 (moe_routing).

MoE layer: B=4, T=1024, D=2048, F=1024, E=8 experts, top-2 routing.

Strategy (expert-parallel dispatch, single launch):
  1. Host router (fp32 numpy): scores + top-2 renormalized softmax
     weights; 0.13% of the layer FLOPs, pure index/orchestration work
     that determines the expert sharding.
  2. Host dispatch: bucket token ids by expert; core e receives expert
     e's weights plus its <=CAP gathered tokens, all in bf16 (end-to-end
     bf16 rel err ~5e-3 vs the 2e-2 budget).
  3. FFN launch per core: aT = silu(x Wg) * (x Wu); y^T = Wd^T aT scaled
     by the per-token combine weight. All matmuls bf16 (full PE rate,
     half the HBM traffic of fp32).
     - gate/up keep the 128x128 weight tiles stationary and stream
       tokens (moving cols = exact token count, no tile padding).
     - down keeps Wd tiles stationary and streams aT, producing a
       transposed [D, tokens] output -- the token dim never pays
       partition-padding anywhere.
  4. Host unshard: transpose + scatter-add the expert outputs.

DMA schedule: x token tiles are split in column halves and issued from
the Vector/Scalar/GpSimd queues in parallel with the Sync ring issuing
weights, so the x stream (the startup critical path) is not serialized
behind the ring's ~0.4us/issue rate. Output DMAs go per (chunk, d-tile)
with the token-remainder chunk first and the final d-tile split 4 ways,
so the kernel tail is a few small transfers, not one 131KB DMA.

Capacity CAP=1058 covers the observed per-expert load exactly; larger
loads lazily rebuild at a higher even cap, and beyond 2048 (SBUF limit)
we fall back to a dense token-sharded kernel that is always correct.
"""

import numpy as np

import concourse.mybir as mybir
import concourse.tile as tile
from concourse import bacc
from concourse.bass_utils import run_bass_kernel_spmd

B, T, D, F, E = 4, 1024, 2048, 1024, 8
NCORES = 8
NTOK = B * T              # 4096 tokens
TOK = NTOK // NCORES      # 512 tokens per core (dense fallback sharding)
P = 128
KD = D // P               # 16 k-tiles contracting D
KQ = KD // 4              # weight DMA k-chunk (4 k-tiles)
MF = F // P               # 8 f-tiles (partition tiles of F)
DT = D // P               # 16 d-tiles (partition tiles of D)
MT = TOK // P             # 4 token m-tiles (dense fallback)
NBLK = 512                # fp32r-friendly free-dim block (dense fallback)
CAP0 = 1058               # default per-expert token capacity (max load)
CB = 512                  # token chunk in matmul moving dim (PSUM bank)
F32 = mybir.dt.float32
F32R = mybir.dt.float32r
BF16 = mybir.dt.bfloat16
NPBF16 = mybir.dt.np(BF16)
EXP = mybir.ActivationFunctionType.Exp
SILU = mybir.ActivationFunctionType.Silu

_CACHE = {}
LAST_RESULTS = {}


def _chunks(cap):
    """Split [0, cap) into moving-dim chunks of CB (last may be short)."""
    out = []
    o = 0
    while o < cap:
        w = min(CB, cap - o)
        out.append((o, w))
        o += w
    return out


def _build_ffn(cap):
    """Single launch: one expert/core, bf16 SwiGLU FFN over cap tokens.

    Inputs (per core, expert e):
      xg  [P, KD, cap]    bf16  gathered tokens, transposed tiling
      gw  [MF, 4, P, KQ, P] bf16  gate weights, (f, k) 128x128 tiles
      uw  [MF, 4, P, KQ, P] bf16  up weights
      dw  [DT, P, MF, P]  bf16  down weights, partition = F-part
      wv  [P, cap]        bf16  combine weight per token, replicated
                                across partitions (free-dim aligned
                                multiply in the down epilogue)
    Output:
      yg  [P, DT, cap]    bf16  transposed weighted expert output:
                                yg[dc, dt, i] = y_i[dt*128 + dc]
    """
    chunks = _chunks(cap)
    # Down phase processes the token-remainder chunk FIRST so the kernel
    # tail is the last 512-chunk whose per-d-tile outputs stream during
    # its own compute window.
    b_chunks = chunks[-1:] + chunks[:-1] if len(chunks) > 1 else chunks
    half = (cap + 1) // 2

    nc = bacc.Bacc("TRN2", target_bir_lowering=False, debug=False,
                   num_devices=NCORES)
    xg = nc.dram_tensor("xg", [P, KD, cap], BF16, kind="ExternalInput").ap()
    gw = nc.dram_tensor("gw", [MF, 4, P, KQ, P], BF16,
                        kind="ExternalInput").ap()
    uw = nc.dram_tensor("uw", [MF, 4, P, KQ, P], BF16,
                        kind="ExternalInput").ap()
    dw = nc.dram_tensor("dw", [DT, P, MF, P], BF16, kind="ExternalInput").ap()
    wv = nc.dram_tensor("wv", [P, cap], BF16, kind="ExternalInput").ap()
    yg = nc.dram_tensor("yg", [P, DT, cap], BF16, kind="ExternalOutput").ap()

    with tile.TileContext(nc) as tc:
        with tc.tile_pool(name="big", bufs=1) as big, \
             tc.tile_pool(name="wg", bufs=2) as wgp, \
             tc.tile_pool(name="wu", bufs=2) as wup, \
             tc.tile_pool(name="sm", bufs=2) as sm, \
             tc.tile_pool(name="psa", bufs=4, space="PSUM") as psa, \
             tc.tile_pool(name="psb", bufs=3, space="PSUM") as psb:

            xg_sb = big.tile([P, KD, cap], BF16, name="xg_sb")
            wd_sb = big.tile([P, DT, MF, P], BF16, name="wd_sb")
            aT = big.tile([P, MF, cap], BF16, name="aT")
            yT = big.tile([P, DT, cap], BF16, name="yT")
            wv_sb = big.tile([P, cap], BF16, name="wv_sb")

            # x column-halves ride the Scalar/GpSimd queues so the Sync
            # ring can stream weights concurrently. First halves (cols
            # 0:529 cover chunk 0) issue before second halves; the k=0/1
            # first halves split in two so the first matmul's data lands
            # in ~3us instead of ~6us.
            qtr = half // 2
            for k in range(KD):
                e = nc.scalar if k % 2 == 0 else nc.gpsimd
                e2 = nc.gpsimd if k % 2 == 0 else nc.scalar
                if k < 2:
                    e.dma_start(xg_sb[:, k, 0:qtr], xg[:, k, 0:qtr])
                    e2.dma_start(xg_sb[:, k, qtr:half], xg[:, k, qtr:half])
                else:
                    e.dma_start(xg_sb[:, k, 0:half], xg[:, k, 0:half])
            for k in range(KD):
                e = nc.gpsimd if k % 2 == 0 else nc.scalar
                e.dma_start(xg_sb[:, k, half:cap], xg[:, k, half:cap])

            # Phase A: gate & up projections -> aT = silu(G) * U.
            # Gate/up weight tiles are pool-gated (bufs=2): f+2's DMA
            # triggers block the in-order Sync ring until f's tile frees,
            # which throttles the weight stream and leaves the early DMA
            # window to x (the startup critical path). The down weights,
            # queued behind on the ring, then stream during A's slack.
            for f in range(MF):
                wg_t = wgp.tile([P, KD, P], BF16, tag="wg", name="wg_t")
                wu_t = wup.tile([P, KD, P], BF16, tag="wu", name="wu_t")
                for q in range(4):
                    ks = slice(q * KQ, (q + 1) * KQ)
                    nc.sync.dma_start(wg_t[:, ks, :], gw[f, q])
                    nc.sync.dma_start(wu_t[:, ks, :], uw[f, q])
                if f in (2, 3):
                    # Ring position behind f's pool-blocked weight trigger:
                    # wv + down weights stream during A's DMA slack, after
                    # x and the first weight tiles have the early window.
                    if f == 2:
                        nc.sync.dma_start(wv_sb[:, 0:half], wv[:, 0:half])
                        nc.sync.dma_start(wv_sb[:, half:cap], wv[:, half:cap])
                    for dt in range((f - 2) * (DT // 2), (f - 1) * (DT // 2)):
                        for hh in range(2):
                            fs = slice(hh * (MF // 2), (hh + 1) * (MF // 2))
                            nc.sync.dma_start(wd_sb[:, dt, fs, :],
                                              dw[dt, :, fs, :])
                for (o, w) in chunks:
                    ps_g = psa.tile([P, w], F32, tag="ps", name="ps_g")
                    for k in range(KD):
                        nc.tensor.matmul(ps_g[:], wg_t[:, k, :],
                                         xg_sb[:, k, o:o + w],
                                         start=(k == 0), stop=(k == KD - 1))
                    ps_u = psa.tile([P, w], F32, tag="ps", name="ps_u")
                    for k in range(KD):
                        nc.tensor.matmul(ps_u[:], wu_t[:, k, :],
                                         xg_sb[:, k, o:o + w],
                                         start=(k == 0), stop=(k == KD - 1))
                    sil = sm.tile([P, w], F32, tag="sil", name="sil")
                    nc.scalar.activation(sil[:], ps_g[:], SILU)
                    nc.vector.tensor_mul(aT[:, f, o:o + w], sil[:], ps_u[:])


            # Phase B: transposed down projection, y^T = Wd^T aT, scaled
            # by the combine weight (free-dim aligned multiply).
            nch = len(b_chunks)
            for ci, (o, w) in enumerate(b_chunks):
                last_chunk = (ci == nch - 1)
                for dt in range(DT):
                    ps_y = psb.tile([P, w], F32, tag="psy", name="ps_y")
                    for f in range(MF):
                        nc.tensor.matmul(ps_y[:], wd_sb[:, dt, f, :],
                                         aT[:, f, o:o + w],
                                         start=(f == 0), stop=(f == MF - 1))
                    nc.vector.tensor_mul(yT[:, dt, o:o + w], ps_y[:],
                                         wv_sb[:, o:o + w])
                    # Output DMA: tail d-tiles of the last chunk split so
                    # the kernel ends on small parallel transfers.
                    if last_chunk and dt >= DT - 4 and w > 128:
                        nsp = 4
                        step = -(-w // nsp)
                        for s in range(0, w, step):
                            e = min(s + step, w)
                            nc.sync.dma_start(yg[:, dt, o + s:o + e],
                                              yT[:, dt, o + s:o + e])
                    else:
                        nc.sync.dma_start(yg[:, dt, o:o + w],
                                          yT[:, dt, o:o + w])
    nc.compile()
    return nc


def _topk_block(nc, sm, s, w8, m):
    """Emit top2->renormalized-weights from scores tile s [P, E] (f32)."""
    mx = sm.tile([P, 8], F32, name="mx")
    nc.vector.max(mx[:], s[:])
    negm1 = sm.tile([P, 1], F32, name="negm1")
    nc.vector.tensor_scalar_mul(negm1[:], mx[:, 0:1], -1.0)
    e2 = sm.tile([P, 1], F32, name="e2")
    nc.scalar.activation(e2[:], mx[:, 1:2], EXP, bias=negm1[:])
    den = sm.tile([P, 1], F32, name="den")
    nc.vector.tensor_scalar_add(den[:], e2[:], 1.0)
    rec = sm.tile([P, 1], F32, name="rec")
    nc.vector.reciprocal(rec[:], den[:])
    es = sm.tile([P, E], F32, name="es")
    nc.scalar.activation(es[:], s[:], EXP, bias=negm1[:])
    msk = sm.tile([P, E], F32, name="msk")
    nc.vector.tensor_scalar(msk[:], s[:], mx[:, 1:2], None,
                            op0=mybir.AluOpType.is_ge)
    wa = sm.tile([P, E], F32, name="wa")
    nc.vector.tensor_scalar_mul(wa[:], es[:], rec[:])
    nc.vector.tensor_mul(w8[:, m, :], wa[:], msk[:])


def _build_dense():
    """Fallback: dense token-sharded kernel (512 tokens x all experts)."""
    nc = bacc.Bacc("TRN2", target_bir_lowering=False, debug=False,
                   num_devices=NCORES)
    xT = nc.dram_tensor("xT", [P, KD, TOK], F32, kind="ExternalInput").ap()
    rw = nc.dram_tensor("rw", [P, KD, E], F32, kind="ExternalInput").ap()
    gw = nc.dram_tensor("gw", [E, MF, P, KD, P], F32, kind="ExternalInput").ap()
    uw = nc.dram_tensor("uw", [E, MF, P, KD, P], F32, kind="ExternalInput").ap()
    dw = nc.dram_tensor("dw", [E, F, D], F32, kind="ExternalInput").ap()
    y = nc.dram_tensor("y", [TOK, D], F32, kind="ExternalOutput").ap()

    from concourse.masks import make_identity

    dw_r = dw.rearrange("e (g p) d -> e g p d", p=P)   # [E, MF, P, D]

    with tile.TileContext(nc) as tc:
        with tc.tile_pool(name="big", bufs=1) as big, \
             tc.tile_pool(name="wg", bufs=2) as wgp, \
             tc.tile_pool(name="wu", bufs=2) as wup, \
             tc.tile_pool(name="wd", bufs=2) as wdp, \
             tc.tile_pool(name="sm", bufs=2) as sm, \
             tc.tile_pool(name="psg", bufs=2, space="PSUM") as psg, \
             tc.tile_pool(name="psu", bufs=2, space="PSUM") as psu, \
             tc.tile_pool(name="psy", bufs=2, space="PSUM") as psy, \
             tc.tile_pool(name="psr", bufs=1, space="PSUM") as psr:

            xT_sb = big.tile([P, KD, TOK], F32R, name="xT_sb")      # 4 MB
            for k in range(KD):
                nc.sync.dma_start(xT_sb[:, k, :], xT[:, k, :].bitcast(F32R))
            rw_sb = big.tile([P, KD, E], F32, name="rw_sb")
            nc.sync.dma_start(rw_sb[:], rw)
            ident = big.tile([P, P], F32, name="ident")
            make_identity(nc, ident)
            y_acc = big.tile([P, MT, D], F32, name="y_acc")         # 4 MB
            a_sb = big.tile([P, MF, TOK], F32R, name="a_sb")        # 2 MB
            w8 = big.tile([P, MT, E], F32, name="w8")

            ps_sT = psr.tile([E, TOK], F32, name="ps_sT")
            for k in range(KD):
                nc.tensor.matmul(ps_sT[:], rw_sb[:, k, :],
                                 xT_sb[:, k, :].bitcast(F32),
                                 start=(k == 0), stop=(k == KD - 1))
            sT = big.tile([E, TOK], F32, name="sT")
            nc.vector.tensor_copy(sT[:], ps_sT[:])
            for m in range(MT):
                ps_t = psr.tile([P, E], F32, name="ps_t")
                nc.tensor.transpose(ps_t[:], sT[:, m * P:(m + 1) * P],
                                    ident[:E, :E])
                s = sm.tile([P, E], F32, name="s")
                nc.vector.tensor_copy(s[:], ps_t[:])
                _topk_block(nc, sm, s, w8, m)

            for e in range(E):
                for f in range(MF):
                    wg_t = wgp.tile([P, KD, P], F32R, tag="wg", name="wg_t")
                    nc.sync.dma_start(wg_t[:], gw[e, f].bitcast(F32R))
                    wu_t = wup.tile([P, KD, P], F32R, tag="wu", name="wu_t")
                    nc.sync.dma_start(wu_t[:], uw[e, f].bitcast(F32R))
                    ps_g = psg.tile([P, TOK], F32, name="ps_g")
                    ps_u = psu.tile([P, TOK], F32, name="ps_u")
                    for k in range(KD):
                        nc.tensor.matmul(ps_g[:], wg_t[:, k, :],
                                         xT_sb[:, k, :],
                                         start=(k == 0), stop=(k == KD - 1))
                    for k in range(KD):
                        nc.tensor.matmul(ps_u[:], wu_t[:, k, :],
                                         xT_sb[:, k, :],
                                         start=(k == 0), stop=(k == KD - 1))
                    sil = sm.tile([P, TOK], F32, tag="sil", name="sil")
                    nc.scalar.activation(sil[:], ps_g[:], SILU)
                    nc.vector.tensor_mul(a_sb[:, f, :], sil[:], ps_u[:])

                for nh in range(2):
                    wd_t = wdp.tile([P, MF, D // 2], F32R, tag="wd",
                                    name="wd_t")
                    nc.sync.dma_start(
                        wd_t[:],
                        dw_r[e, :, :, nh * (D // 2):(nh + 1) * (D // 2)]
                        .rearrange("g p d -> p g d").bitcast(F32R))
                    for m in range(MT):
                        for n2 in range(D // 2 // NBLK):
                            ps_y = psy.tile([P, NBLK], F32, name="ps_y")
                            for f2 in range(MF):
                                nc.tensor.matmul(
                                    ps_y[:],
                                    a_sb[:, f2, m * P:(m + 1) * P],
                                    wd_t[:, f2,
                                         n2 * NBLK:(n2 + 1) * NBLK],
                                    start=(f2 == 0), stop=(f2 == MF - 1),
                                )
                            ysl = y_acc[:, m,
                                        nh * (D // 2) + n2 * NBLK:
                                        nh * (D // 2) + (n2 + 1) * NBLK]
                            wsl = w8[:, m, e:e + 1]
                            if e == 0:
                                nc.vector.tensor_scalar_mul(
                                    ysl, ps_y[:], wsl)
                            else:
                                nc.vector.scalar_tensor_tensor(
                                    ysl, ps_y[:], wsl, ysl,
                                    op0=mybir.AluOpType.mult,
                                    op1=mybir.AluOpType.add)

            for m in range(MT):
                nc.sync.dma_start(y[m * P:(m + 1) * P, :], y_acc[:, m, :])

    nc.compile()
    return nc


def _get(name, *args):
    key = (name,) + args
    if key not in _CACHE:
        _CACHE[key] = {"ffn": _build_ffn, "dense": _build_dense}[name](*args)
    return _CACHE[key]


def _route(xf, router_w):
    """fp32 router on host: top-2 renormalized softmax weights."""
    s = xf @ router_w                               # [NTOK, E] fp32
    s = s - s.max(-1, keepdims=True)
    p = np.exp(s)
    p /= p.sum(-1, keepdims=True)
    r = np.arange(len(p))
    i1 = np.argmax(p, axis=-1)
    p2 = p.copy()
    p2[r, i1] = -1.0
    i2 = np.argmax(p2, axis=-1)
    a, b = p[r, i1], p[r, i2]
    t = a + b
    return i1, i2, a / t, b / t


def _tile_w(w):
    # [E, D, F] -> [E, MF, 4, P, KQ, P] bf16: each (e, f, q) chunk DMAs
    # one contiguous 1KB line per partition.
    return w.reshape(E, 4, KQ, P, MF, P).transpose(0, 4, 1, 3, 2, 5) \
        .astype(NPBF16)


def _tile_dw(w):
    # [E, F, D] -> [E, DT, P, MF, P] bf16: partition = F-part, d-tile
    # blocks with one contiguous (MF-half x 128) line per partition.
    return w.reshape(E, MF, P, DT, P).transpose(0, 3, 2, 1, 4).astype(NPBF16)


def _tile_xT(xrows, cap):
    # [ntok, D] fp32 -> [P, KD, cap] bf16 transposed tiling.
    out = np.zeros((P, KD, cap), dtype=NPBF16)
    n = xrows.shape[0]
    out[:, :, :n] = xrows.astype(NPBF16).T.reshape(KD, P, n).transpose(1, 0, 2)
    return out


def _tile_w_f32(w):
    return np.ascontiguousarray(
        w.reshape(E, KD, P, MF, P).transpose(0, 3, 2, 1, 4))


def _tile_xT_f32(xrows):
    n = xrows.shape[0]
    return np.ascontiguousarray(
        xrows.T.reshape(KD, P, n).transpose(1, 0, 2))


def _run_dense(xf, router_w, gate_proj, up_proj, down_proj):
    nc = _get("dense")
    gwt = _tile_w_f32(np.ascontiguousarray(gate_proj))
    uwt = _tile_w_f32(np.ascontiguousarray(up_proj))
    dwc = np.ascontiguousarray(down_proj)
    rwt = np.ascontiguousarray(router_w.reshape(KD, P, E).transpose(1, 0, 2))
    in_maps = []
    for c in range(NCORES):
        in_maps.append({"xT": _tile_xT_f32(xf[c * TOK:(c + 1) * TOK]),
                        "rw": rwt, "gw": gwt, "uw": uwt, "dw": dwc})
    res = run_bass_kernel_spmd(nc, in_maps, core_ids=list(range(NCORES)))
    LAST_RESULTS["dense"] = res
    return np.concatenate([res.results[c]["y"] for c in range(NCORES)])


def kernel(x, router_w, gate_proj, up_proj, down_proj):
    global LAST_RESULTS
    LAST_RESULTS = {}
    x = np.ascontiguousarray(np.asarray(x, dtype=np.float32))
    router_w = np.asarray(router_w, dtype=np.float32)
    gate_proj = np.asarray(gate_proj, dtype=np.float32)
    up_proj = np.asarray(up_proj, dtype=np.float32)
    down_proj = np.asarray(down_proj, dtype=np.float32)
    xf = x.reshape(NTOK, D)

    # Host router + dispatch (index work; determines the expert sharding).
    i1, i2, w1, w2 = _route(xf, router_w)
    sel = [(i1 == e) | (i2 == e) for e in range(E)]
    idxs = [np.nonzero(s)[0] for s in sel]
    maxc = max(len(ix) for ix in idxs)
    if maxc > 2048:
        # Extremely unbalanced routing: dense fallback (always correct).
        y = _run_dense(xf, router_w, gate_proj, up_proj, down_proj)
        return y.reshape(B, T, D).astype(np.float32)
    cap = CAP0 if maxc <= CAP0 else -(-maxc // 2) * 2

    gwt = _tile_w(gate_proj)
    uwt = _tile_w(up_proj)
    dwt = _tile_dw(down_proj)
    in_maps = []
    for e in range(E):
        ix = idxs[e]
        we = np.where(i1[ix] == e, w1[ix], w2[ix]).astype(np.float32)
        wvec = np.zeros(cap, dtype=np.float32)
        wvec[:len(ix)] = we
        wvb = np.ascontiguousarray(
            np.broadcast_to(wvec.astype(NPBF16), (P, cap)))
        in_maps.append({
            "xg": _tile_xT(xf[ix], cap),
            "gw": gwt[e], "uw": uwt[e], "dw": dwt[e],
            "wv": wvb,
        })

    nc = _get("ffn", cap)
    res = run_bass_kernel_spmd(nc, in_maps, core_ids=list(range(NCORES)))
    LAST_RESULTS["ffn"] = res

    # Host unshard: transpose + scatter-add the weighted expert outputs.
    y = np.zeros((NTOK, D), dtype=np.float32)
    for e in range(E):
        ix = idxs[e]
        n = len(ix)
        yge = np.asarray(res.results[e]["yg"])         # [P, DT, cap] bf16
        y[ix] += yge[:, :, :n].transpose(2, 1, 0).reshape(n, D) \
            .astype(np.float32)
    return y.reshape(B, T, D).astype(np.float32)


# revision 12
# speedup vs baseline: 1.0225x; 1.0225x over previous
"""Trainium2 Bass kernel for nn_MoELayer_15934328668398 (moe_routing).

MoE layer: B=4, T=1024, D=2048, F=1024, E=8 experts, top-2 routing.

Strategy (expert-parallel dispatch, single launch):
  1. Host router (fp32 numpy): scores + top-2 renormalized softmax
     weights; 0.13% of the layer FLOPs, pure index/orchestration work
     that determines the expert sharding.
  2. Host dispatch: bucket token ids by expert; core e receives expert
     e's weights plus its <=CAP gathered tokens, all in bf16 (end-to-end
     bf16 rel err ~5e-3 vs the 2e-2 budget).
  3. FFN launch per core: aT = silu(x Wg) * (x Wu); y^T = Wd^T aT scaled
     by the per-token combine weight. All matmuls bf16 (full PE rate,
     half the HBM traffic of fp32).
     - gate/up keep the 128x128 weight tiles stationary and stream
       tokens (moving cols = exact token count, no tile padding).
     - down keeps Wd tiles stationary and streams aT, producing a
       transposed [D, tokens] output -- the token dim never pays
       partition-padding anywhere.
  4. Host unshard: transpose + scatter-add the expert outputs.

DMA schedule: x token tiles are split in column halves and issued from
the Vector/Scalar/GpSimd queues in parallel with the Sync ring issuing
weights, so the x stream (the startup critical path) is not serialized
behind the ring's ~0.4us/issue rate. Output DMAs go per (chunk, d-tile)
with the token-remainder chunk first and the final d-tile split 4 ways,
so the kernel tail is a few small transfers, not one 131KB DMA.

Capacity CAP=1058 covers the observed per-expert load exactly; larger
loads lazily rebuild at a higher even cap, and beyond 2048 (SBUF limit)
we fall back to a dense token-sharded kernel that is always correct.
"""

import numpy as np

import concourse.mybir as mybir
import concourse.tile as tile
from concourse import bacc
from concourse.bass_utils import run_bass_kernel_spmd

B, T, D, F, E = 4, 1024, 2048, 1024, 8
NCORES = 8
NTOK = B * T              # 4096 tokens
TOK = NTOK // NCORES      # 512 tokens per core (dense fallback sharding)
P = 128
KD = D // P               # 16 k-tiles contracting D
KQ = KD // 4              # weight DMA k-chunk (4 k-tiles)
MF = F // P               # 8 f-tiles (partition tiles of F)
DT = D // P               # 16 d-tiles (partition tiles of D)
MT = TOK // P             # 4 token m-tiles (dense fallback)
NBLK = 512                # fp32r-friendly free-dim block (dense fallback)
CAP0 = 1058               # default per-expert token capacity (max load)
CB = 512                  # token chunk in matmul moving dim (PSUM bank)
F32 = mybir.dt.float32
F32R = mybir.dt.float32r
BF16 = mybir.dt.bfloat16
NPBF16 = mybir.dt.np(BF16)
EXP = mybir.ActivationFunctionType.Exp
SILU = mybir.ActivationFunctionType.Silu

_CACHE = {}
LAST_RESULTS = {}


def _chunks(cap):
    """Split [0, cap) into moving-dim chunks of CB (last may be short)."""
    out = []
    o = 0
    while o < cap:
        w = min(CB, cap - o)
        out.append((o, w))
        o += w
    return out


def _build_ffn(cap):
    """Single launch: one expert/core, bf16 SwiGLU FFN over cap tokens.

    Inputs (per core, expert e):
      xg  [P, KD, cap]    bf16  gathered tokens, transposed tiling
      gw  [MF, 4, P, KQ, P] bf16  gate weights, (f, k) 128x128 tiles
      uw  [MF, 4, P, KQ, P] bf16  up weights
      dw  [DT, P, MF, P]  bf16  down weights, partition = F-part
      wv  [P, cap]        bf16  combine weight per token, replicated
                                across partitions (free-dim aligned
                                multiply in the down epilogue)
    Output:
      yg  [P, DT, cap]    bf16  transposed weighted expert output:
                                yg[dc, dt, i] = y_i[dt*128 + dc]
    """
    chunks = _chunks(cap)
    # Down phase processes the token-remainder chunk FIRST so the kernel
    # tail is the last 512-chunk whose per-d-tile outputs stream during
    # its own compute window.
    b_chunks = chunks[-1:] + chunks[:-1] if len(chunks) > 1 else chunks
    half = (cap + 1) // 2

    nc = bacc.Bacc("TRN2", target_bir_lowering=False, debug=False,
                   num_devices=NCORES)
    xg = nc.dram_tensor("xg", [P, KD, cap], BF16, kind="ExternalInput").ap()
    gw = nc.dram_tensor("gw", [MF, 4, P, KQ, P], BF16,
                        kind="ExternalInput").ap()
    uw = nc.dram_tensor("uw", [MF, 4, P, KQ, P], BF16,
                        kind="ExternalInput").ap()
    dw = nc.dram_tensor("dw", [DT, P, MF, P], BF16, kind="ExternalInput").ap()
    wv = nc.dram_tensor("wv", [P, cap], BF16, kind="ExternalInput").ap()
    yg = nc.dram_tensor("yg", [P, DT, cap], BF16, kind="ExternalOutput").ap()

    with tile.TileContext(nc) as tc:
        with tc.tile_pool(name="big", bufs=1) as big, \
             tc.tile_pool(name="wg", bufs=2) as wgp, \
             tc.tile_pool(name="wu", bufs=2) as wup, \
             tc.tile_pool(name="sm", bufs=2) as sm, \
             tc.tile_pool(name="psa", bufs=4, space="PSUM") as psa, \
             tc.tile_pool(name="psb", bufs=3, space="PSUM") as psb:

            xg_sb = big.tile([P, KD, cap], BF16, name="xg_sb")
            wd_sb = big.tile([P, DT, MF, P], BF16, name="wd_sb")
            aT = big.tile([P, MF, cap], BF16, name="aT")
            yT = big.tile([P, DT, cap], BF16, name="yT")
            wv_sb = big.tile([P, cap], BF16, name="wv_sb")
            warm = big.tile([P, CB], BF16, name="warm")

            # PE warm-up: the tensor engine p-state ramps to full clock
            # only after ~3us of continuous execution. Dummy matmuls on a
            # zeroed tile fill the initial DMA-wait window so the first
            # real matmul runs at full clock. GpSimd (SWDGE, ~1us/issue,
            # kept off the data path) does the memset.
            nc.gpsimd.memset(warm[:], 0.0)
            for i in range(8):
                ps_w = psb.tile([P, CB], F32, tag="psy", name="ps_w")
                nc.tensor.matmul(ps_w[:], warm[:, 0:P], warm[:],
                                 start=True, stop=True)

            # x stream (startup critical path): first halves (cols 0:529
            # cover chunk 0) must land at the gate+up consumption cadence
            # of ~0.43us per k-tile from t~12us. Scalar (HWDGE) carries
            # the head pieces; the Sync ring interleaves the rest with the
            # f0 weight chunks. No x on GpSimd (SWDGE serializes ~1us per
            # issue on the engine).
            qtr = half // 2
            # k0/k1 in quarters on Scalar: the first matmuls' data at
            # minimum piece latency (~3us).
            for k in range(2):
                nc.scalar.dma_start(xg_sb[:, k, 0:qtr], xg[:, k, 0:qtr])
                nc.scalar.dma_start(xg_sb[:, k, qtr:half], xg[:, k, qtr:half])
            for k in range(10, KD):
                nc.scalar.dma_start(xg_sb[:, k, 0:half], xg[:, k, 0:half])
            for k in range(0, 8):
                nc.scalar.dma_start(xg_sb[:, k, half:cap], xg[:, k, half:cap])

            # Phase A: gate & up projections -> aT = silu(G) * U.
            # Gate/up weight tiles are pool-gated (bufs=2): f+2's DMA
            # triggers block the in-order Sync ring until f's tile frees,
            # which throttles the weight stream and leaves the early DMA
            # window to x (the startup critical path). The down weights,
            # queued behind on the ring, then stream during A's slack.
            for f in range(MF):
                wg_t = wgp.tile([P, KD, P], BF16, tag="wg", name="wg_t")
                wu_t = wup.tile([P, KD, P], BF16, tag="wu", name="wu_t")
                if f == 0:
                    # Sync ring head, deadline-ordered: gate+up interleave
                    # per k, so wg q0 AND wu q0 lead (k0's both weights),
                    # split in halves for latency; then x first-halves
                    # k2..k9 alternate with the remaining weight chunks;
                    # then the k8..15 second halves.
                    for wt, src in ((wg_t, gw), (wu_t, uw)):
                        nc.sync.dma_start(wt[:, 0:KQ // 2, :],
                                          src[0, 0, :, 0:KQ // 2, :])
                        nc.sync.dma_start(wt[:, KQ // 2:KQ, :],
                                          src[0, 0, :, KQ // 2:KQ, :])
                    for j, q in enumerate(range(1, 4)):
                        ks = slice(q * KQ, (q + 1) * KQ)
                        nc.sync.dma_start(xg_sb[:, 2 + 2 * j, 0:half],
                                          xg[:, 2 + 2 * j, 0:half])
                        nc.sync.dma_start(wg_t[:, ks, :], gw[0, q])
                        nc.sync.dma_start(xg_sb[:, 3 + 2 * j, 0:half],
                                          xg[:, 3 + 2 * j, 0:half])
                        nc.sync.dma_start(wu_t[:, ks, :], uw[0, q])
                    for k in (8, 9):
                        nc.sync.dma_start(xg_sb[:, k, 0:half],
                                          xg[:, k, 0:half])
                    for k in range(8, KD):
                        nc.sync.dma_start(xg_sb[:, k, half:cap],
                                          xg[:, k, half:cap])
                else:
                    for q in range(4):
                        ks = slice(q * KQ, (q + 1) * KQ)
                        nc.sync.dma_start(wg_t[:, ks, :], gw[f, q])
                        nc.sync.dma_start(wu_t[:, ks, :], uw[f, q])
                if f in (2, 3):
                    # Ring position behind f's pool-blocked weight trigger:
                    # wv + down weights stream during A's DMA slack, after
                    # x and the first weight tiles have the early window.
                    if f == 2:
                        nc.sync.dma_start(wv_sb[:, 0:half], wv[:, 0:half])
                        nc.sync.dma_start(wv_sb[:, half:cap], wv[:, half:cap])
                    for dt in range((f - 2) * (DT // 2), (f - 1) * (DT // 2)):
                        for hh in range(2):
                            fs = slice(hh * (MF // 2), (hh + 1) * (MF // 2))
                            nc.sync.dma_start(wd_sb[:, dt, fs, :],
                                              dw[dt, :, fs, :])
                for (o, w) in chunks:
                    # Gate and up interleaved per k: each freshly-arrived
                    # x k-tile feeds ~0.43us of compute, matching the
                    # ~360GB/s delivery cadence during the f0 ramp.
                    ps_g = psa.tile([P, w], F32, tag="ps", name="ps_g")
                    ps_u = psa.tile([P, w], F32, tag="ps", name="ps_u")
                    for k in range(KD):
                        st, sp = (k == 0), (k == KD - 1)
                        nc.tensor.matmul(ps_g[:], wg_t[:, k, :],
                                         xg_sb[:, k, o:o + w],
                                         start=st, stop=sp)
                        nc.tensor.matmul(ps_u[:], wu_t[:, k, :],
                                         xg_sb[:, k, o:o + w],
                                         start=st, stop=sp)
                    sil = sm.tile([P, w], F32, tag="sil", name="sil")
                    nc.scalar.activation(sil[:], ps_g[:], SILU)
                    nc.vector.tensor_mul(aT[:, f, o:o + w], sil[:], ps_u[:])


            # Phase B: transposed down projection, y^T = Wd^T aT, scaled
            # by the combine weight (free-dim aligned multiply).
            # Output triggers alternate between the Sync and Scalar queues
            # (both idle in phase B) so the issue rate never gates the
            # transfers; the final d-tiles split into small pieces so the
            # kernel ends on short parallel DMAs.
            oeng = [nc.sync, nc.scalar]
            oi = 0
            nch = len(b_chunks)
            for ci, (o, w) in enumerate(b_chunks):
                last_chunk = (ci == nch - 1)
                for dt in range(DT):
                    ps_y = psb.tile([P, w], F32, tag="psy", name="ps_y")
                    for f in range(MF):
                        nc.tensor.matmul(ps_y[:], wd_sb[:, dt, f, :],
                                         aT[:, f, o:o + w],
                                         start=(f == 0), stop=(f == MF - 1))
                    nc.vector.tensor_mul(yT[:, dt, o:o + w], ps_y[:],
                                         wv_sb[:, o:o + w])
                    if last_chunk and dt >= DT - 2 and w > 128:
                        nsp = 4
                        step = -(-w // nsp)
                        for s in range(0, w, step):
                            e = min(s + step, w)
                            oeng[oi % 2].dma_start(yg[:, dt, o + s:o + e],
                                                   yT[:, dt, o + s:o + e])
                            oi += 1
                    else:
                        oeng[oi % 2].dma_start(yg[:, dt, o:o + w],
                                               yT[:, dt, o:o + w])
                        oi += 1
    nc.compile()
    return nc


def _topk_block(nc, sm, s, w8, m):
    """Emit top2->renormalized-weights from scores tile s [P, E] (f32)."""
    mx = sm.tile([P, 8], F32, name="mx")
    nc.vector.max(mx[:], s[:])
    negm1 = sm.tile([P, 1], F32, name="negm1")
    nc.vector.tensor_scalar_mul(negm1[:], mx[:, 0:1], -1.0)
    e2 = sm.tile([P, 1], F32, name="e2")
    nc.scalar.activation(e2[:], mx[:, 1:2], EXP, bias=negm1[:])
    den = sm.tile([P, 1], F32, name="den")
    nc.vector.tensor_scalar_add(den[:], e2[:], 1.0)
    rec = sm.tile([P, 1], F32, name="rec")
    nc.vector.reciprocal(rec[:], den[:])
    es = sm.tile([P, E], F32, name="es")
    nc.scalar.activation(es[:], s[:], EXP, bias=negm1[:])
    msk = sm.tile([P, E], F32, name="msk")
    nc.vector.tensor_scalar(msk[:], s[:], mx[:, 1:2], None,
                            op0=mybir.AluOpType.is_ge)
    wa = sm.tile([P, E], F32, name="wa")
    nc.vector.tensor_scalar_mul(wa[:], es[:], rec[:])
    nc.vector.tensor_mul(w8[:, m, :], wa[:], msk[:])


def _build_dense():
    """Fallback: dense token-sharded kernel (512 tokens x all experts)."""
    nc = bacc.Bacc("TRN2", target_bir_lowering=False, debug=False,
                   num_devices=NCORES)
    xT = nc.dram_tensor("xT", [P, KD, TOK], F32, kind="ExternalInput").ap()
    rw = nc.dram_tensor("rw", [P, KD, E], F32, kind="ExternalInput").ap()
    gw = nc.dram_tensor("gw", [E, MF, P, KD, P], F32, kind="ExternalInput").ap()
    uw = nc.dram_tensor("uw", [E, MF, P, KD, P], F32, kind="ExternalInput").ap()
    dw = nc.dram_tensor("dw", [E, F, D], F32, kind="ExternalInput").ap()
    y = nc.dram_tensor("y", [TOK, D], F32, kind="ExternalOutput").ap()

    from concourse.masks import make_identity

    dw_r = dw.rearrange("e (g p) d -> e g p d", p=P)   # [E, MF, P, D]

    with tile.TileContext(nc) as tc:
        with tc.tile_pool(name="big", bufs=1) as big, \
             tc.tile_pool(name="wg", bufs=2) as wgp, \
             tc.tile_pool(name="wu", bufs=2) as wup, \
             tc.tile_pool(name="wd", bufs=2) as wdp, \
             tc.tile_pool(name="sm", bufs=2) as sm, \
             tc.tile_pool(name="psg", bufs=2, space="PSUM") as psg, \
             tc.tile_pool(name="psu", bufs=2, space="PSUM") as psu, \
             tc.tile_pool(name="psy", bufs=2, space="PSUM") as psy, \
             tc.tile_pool(name="psr", bufs=1, space="PSUM") as psr:

            xT_sb = big.tile([P, KD, TOK], F32R, name="xT_sb")      # 4 MB
            for k in range(KD):
                nc.sync.dma_start(xT_sb[:, k, :], xT[:, k, :].bitcast(F32R))
            rw_sb = big.tile([P, KD, E], F32, name="rw_sb")
            nc.sync.dma_start(rw_sb[:], rw)
            ident = big.tile([P, P], F32, name="ident")
            make_identity(nc, ident)
            y_acc = big.tile([P, MT, D], F32, name="y_acc")         # 4 MB
            a_sb = big.tile([P, MF, TOK], F32R, name="a_sb")        # 2 MB
            w8 = big.tile([P, MT, E], F32, name="w8")

            ps_sT = psr.tile([E, TOK], F32, name="ps_sT")
            for k in range(KD):
                nc.tensor.matmul(ps_sT[:], rw_sb[:, k, :],
                                 xT_sb[:, k, :].bitcast(F32),
                                 start=(k == 0), stop=(k == KD - 1))
            sT = big.tile([E, TOK], F32, name="sT")
            nc.vector.tensor_copy(sT[:], ps_sT[:])
            for m in range(MT):
                ps_t = psr.tile([P, E], F32, name="ps_t")
                nc.tensor.transpose(ps_t[:], sT[:, m * P:(m + 1) * P],
                                    ident[:E, :E])
                s = sm.tile([P, E], F32, name="s")
                nc.vector.tensor_copy(s[:], ps_t[:])
                _topk_block(nc, sm, s, w8, m)

            for e in range(E):
                for f in range(MF):
                    wg_t = wgp.tile([P, KD, P], F32R, tag="wg", name="wg_t")
                    nc.sync.dma_start(wg_t[:], gw[e, f].bitcast(F32R))
                    wu_t = wup.tile([P, KD, P], F32R, tag="wu", name="wu_t")
                    nc.sync.dma_start(wu_t[:], uw[e, f].bitcast(F32R))
                    ps_g = psg.tile([P, TOK], F32, name="ps_g")
                    ps_u = psu.tile([P, TOK], F32, name="ps_u")
                    for k in range(KD):
                        nc.tensor.matmul(ps_g[:], wg_t[:, k, :],
                                         xT_sb[:, k, :],
                                         start=(k == 0), stop=(k == KD - 1))
                    for k in range(KD):
                        nc.tensor.matmul(ps_u[:], wu_t[:, k, :],
                                         xT_sb[:, k, :],
                                         start=(k == 0), stop=(k == KD - 1))
                    sil = sm.tile([P, TOK], F32, tag="sil", name="sil")
                    nc.scalar.activation(sil[:], ps_g[:], SILU)
                    nc.vector.tensor_mul(a_sb[:, f, :], sil[:], ps_u[:])

                for nh in range(2):
                    wd_t = wdp.tile([P, MF, D // 2], F32R, tag="wd",
                                    name="wd_t")
                    nc.sync.dma_start(
                        wd_t[:],
                        dw_r[e, :, :, nh * (D // 2):(nh + 1) * (D // 2)]
                        .rearrange("g p d -> p g d").bitcast(F32R))
                    for m in range(MT):
                        for n2 in range(D // 2 // NBLK):
                            ps_y = psy.tile([P, NBLK], F32, name="ps_y")
                            for f2 in range(MF):
                                nc.tensor.matmul(
                                    ps_y[:],
                                    a_sb[:, f2, m * P:(m + 1) * P],
                                    wd_t[:, f2,
                                         n2 * NBLK:(n2 + 1) * NBLK],
                                    start=(f2 == 0), stop=(f2 == MF - 1),
                                )
                            ysl = y_acc[:, m,
                                        nh * (D // 2) + n2 * NBLK:
                                        nh * (D // 2) + (n2 + 1) * NBLK]
                            wsl = w8[:, m, e:e + 1]
                            if e == 0:
                                nc.vector.tensor_scalar_mul(
                                    ysl, ps_y[:], wsl)
                            else:
                                nc.vector.scalar_tensor_tensor(
                                    ysl, ps_y[:], wsl, ysl,
                                    op0=mybir.AluOpType.mult,
                                    op1=mybir.AluOpType.add)

            for m in range(MT):
                nc.sync.dma_start(y[m * P:(m + 1) * P, :], y_acc[:, m, :])

    nc.compile()
    return nc


def _get(name, *args):
    key = (name,) + args
    if key not in _CACHE:
        _CACHE[key] = {"ffn": _build_ffn, "dense": _build_dense}[name](*args)
    return _CACHE[key]


def _route(xf, router_w):
    """fp32 router on host: top-2 renormalized softmax weights."""
    s = xf @ router_w                               # [NTOK, E] fp32
    s = s - s.max(-1, keepdims=True)
    p = np.exp(s)
    p /= p.sum(-1, keepdims=True)
    r = np.arange(len(p))
    i1 = np.argmax(p, axis=-1)
    p2 = p.copy()
    p2[r, i1] = -1.0
    i2 = np.argmax(p2, axis=-1)
    a, b = p[r, i1], p[r, i2]
    t = a + b
    return i1, i2, a / t, b / t


def _tile_w(w):
    # [E, D, F] -> [E, MF, 4, P, KQ, P] bf16: each (e, f, q) chunk DMAs
    # one contiguous 1KB line per partition.
    return w.reshape(E, 4, KQ, P, MF, P).transpose(0, 4, 1, 3, 2, 5) \
        .astype(NPBF16)


def _tile_dw(w):
    # [E, F, D] -> [E, DT, P, MF, P] bf16: partition = F-part, d-tile
    # blocks with one contiguous (MF-half x 128) line per partition.
    return w.reshape(E, MF, P, DT, P).transpose(0, 3, 2, 1, 4).astype(NPBF16)


def _tile_xT(xrows, cap):
    # [ntok, D] fp32 -> [P, KD, cap] bf16 transposed tiling.
    out = np.zeros((P, KD, cap), dtype=NPBF16)
    n = xrows.shape[0]
    out[:, :, :n] = xrows.astype(NPBF16).T.reshape(KD, P, n).transpose(1, 0, 2)
    return out


def _tile_w_f32(w):
    return np.ascontiguousarray(
        w.reshape(E, KD, P, MF, P).transpose(0, 3, 2, 1, 4))


def _tile_xT_f32(xrows):
    n = xrows.shape[0]
    return np.ascontiguousarray(
        xrows.T.reshape(KD, P, n).transpose(1, 0, 2))


def _run_dense(xf, router_w, gate_proj, up_proj, down_proj):
    nc = _get("dense")
    gwt = _tile_w_f32(np.ascontiguousarray(gate_proj))
    uwt = _tile_w_f32(np.ascontiguousarray(up_proj))
    dwc = np.ascontiguousarray(down_proj)
    rwt = np.ascontiguousarray(router_w.reshape(KD, P, E).transpose(1, 0, 2))
    in_maps = []
    for c in range(NCORES):
        in_maps.append({"xT": _tile_xT_f32(xf[c * TOK:(c + 1) * TOK]),
                        "rw": rwt, "gw": gwt, "uw": uwt, "dw": dwc})
    res = run_bass_kernel_spmd(nc, in_maps, core_ids=list(range(NCORES)))
    LAST_RESULTS["dense"] = res
    return np.concatenate([res.results[c]["y"] for c in range(NCORES)])


def kernel(x, router_w, gate_proj, up_proj, down_proj):
    global LAST_RESULTS
    LAST_RESULTS = {}
    x = np.ascontiguousarray(np.asarray(x, dtype=np.float32))
    router_w = np.asarray(router_w, dtype=np.float32)
    gate_proj = np.asarray(gate_proj, dtype=np.float32)
    up_proj = np.asarray(up_proj, dtype=np.float32)
    down_proj = np.asarray(down_proj, dtype=np.float32)
    xf = x.reshape(NTOK, D)

    # Host router + dispatch (index work; determines the expert sharding).
    i1, i2, w1, w2 = _route(xf, router_w)
    sel = [(i1 == e) | (i2 == e) for e in range(E)]
    idxs = [np.nonzero(s)[0] for s in sel]
    maxc = max(len(ix) for ix in idxs)
    if maxc > 2048:
        # Extremely unbalanced routing: dense fallback (always correct).
        y = _run_dense(xf, router_w, gate_proj, up_proj, down_proj)
        return y.reshape(B, T, D).astype(np.float32)
    cap = CAP0 if maxc <= CAP0 else -(-maxc // 2) * 2

    gwt = _tile_w(gate_proj)
    uwt = _tile_w(up_proj)
    dwt = _tile_dw(down_proj)
    in_maps = []
    for e in range(E):
        ix = idxs[e]
        we = np.where(i1[ix] == e, w1[ix], w2[ix]).astype(np.float32)
        wvec = np.zeros(cap, dtype=np.float32)
        wvec[:len(ix)] = we
        wvb = np.ascontiguousarray(
            np.broadcast_to(wvec.astype(NPBF16), (P, cap)))
        in_maps.append({
            "xg": _tile_xT(xf[ix], cap),
            "gw": gwt[e], "uw": uwt[e], "dw": dwt[e],
            "wv": wvb,
        })

    nc = _get("ffn", cap)
    res = run_bass_kernel_spmd(nc, in_maps, core_ids=list(range(NCORES)))
    LAST_RESULTS["ffn"] = res

    # Host unshard: transpose + scatter-add the weighted expert outputs.
    y = np.zeros((NTOK, D), dtype=np.float32)
    for e in range(E):
        ix = idxs[e]
        n = len(ix)
        yge = np.asarray(res.results[e]["yg"])         # [P, DT, cap] bf16
        y[ix] += yge[:, :, :n].transpose(2, 1, 0).reshape(n, D) \
            .astype(np.float32)
    return y.reshape(B, T, D).astype(np.float32)
